# revision 17
# baseline (speedup 1.0000x reference)
"""Trainium2 Bass kernel for top-2 MoE (nn_ExpertMemory).

Model (reference semantics):
    logits = x @ gate_w + gate_b          # (N, E)
    probs  = softmax(logits)
    gates, idx = top_k(probs, 2)
    out[n] = sum_k gates[n,k] * (relu(x[n] @ w1[e] + b1[e]) @ w2[e] + b2[e]),
             e = idx[n,k]
(The reference runs every expert densely, but combine weights are zero off
the top-2, so routed computation is mathematically identical.)

Strategy: data-parallel over tokens across 8 NeuronCores (1024 tokens each).
Each core, fully on device:
  1. gate matmul + softmax + top-2 (max8/max_index) on its tokens
  2. expert-sorted slot assignment via triangular-matmul prefix sums
  3. token->slot metadata staged through a small DRAM scatter (indirect DMA)
  4. per-expert MLP in a C-major (transposed) domain: activations gathered
     along the free axis with gpsimd ap_gather, fp32r matmuls at full PE rate
  5. gate scaling folded into the expert-output write; combine = two
     free-axis gathers + add; output written C-major, host transposes back.
Slot capacities are specialized per run from a host-side replica of the
routing (inputs only), with margin; the device computes everything itself.
"""

import numpy as np
from contextlib import ExitStack

import concourse.bass as bass
import concourse.tile as tile
import concourse.mybir as mybir
from concourse import bacc

dt = mybir.dt
AF = mybir.ActivationFunctionType
ALU = mybir.AluOpType
AX = mybir.AxisListType

P = 128

# problem constants
B, T, C, E, H, TOPK = 4, 2048, 1024, 8, 2048, 2
NCORE = 8
NTOK = B * T // NCORE  # tokens per core


def _mlp_tiles(s):
    """Split a slot range of size s into moving-dim tiles <= 512, preferring
    every tile >= 256 (fp32r full-rate threshold)."""
    out = []
    off = 0
    rem = s
    while rem > 0:
        if rem > 512:
            if rem < 768:  # split near-evenly, both >= 256
                w = (rem // 2 + 15) // 16 * 16
            else:
                w = 512
        else:
            w = rem
        out.append((off, w))
        off += w
        rem -= w
    return out




def build_program(S, ntok=NTOK, c=C, h=H, e=E, level=9):
    nc = _build(S, ntok=ntok, c=c, h=h, e=e, level=level)
    nc.compile()
    return nc


def _build(S, ntok=NTOK, c=C, h=H, e=E, level=9):
    """Build the SPMD Tile program. S: per-expert slot capacities
    (multiples of 16, sum multiple of 128)."""
    TCH = ntok // P   # token chunks
    CK = c // P       # contraction chunks for layer 1 / output chunks
    HK = h // P       # hidden chunks
    NH = ntok // 512  # 512-wide token halves
    NSLOT = int(sum(S))
    base = np.concatenate([[0], np.cumsum(S)]).astype(np.int64)
    assert NSLOT % P == 0 and all(s % 16 == 0 for s in S)
    assert NSLOT <= 32000  # int16 gather indices

    nc = bacc.Bacc("TRN2", target_bir_lowering=False, debug=False)

    f32, bf16 = dt.float32, dt.bfloat16
    xT = nc.dram_tensor("xT", [c, ntok], f32, kind="ExternalInput").ap()
    gw = nc.dram_tensor("gw", [c, e], f32, kind="ExternalInput").ap()
    gb = nc.dram_tensor("gb", [e, 1], f32, kind="ExternalInput").ap()
    w1 = nc.dram_tensor("w1", [e, c, h], bf16, kind="ExternalInput").ap()
    b1 = nc.dram_tensor("b1", [e, h, 1], f32, kind="ExternalInput").ap()
    w2 = nc.dram_tensor("w2", [e, h, c], bf16, kind="ExternalInput").ap()
    b2 = nc.dram_tensor("b2", [e, c, 1], f32, kind="ExternalInput").ap()
    tri = nc.dram_tensor("tri", [P, P], f32, kind="ExternalInput").ap()
    id8 = nc.dram_tensor("id8", [e, e], f32, kind="ExternalInput").ap()
    ebase = nc.dram_tensor("ebase", [1, e], f32, kind="ExternalInput").ap()
    yT = nc.dram_tensor("yT", [c, ntok], f32, kind="ExternalOutput").ap()

    meta = nc.dram_tensor("meta", [NSLOT, 2], f32).ap()      # (tok, gate)
    slotbuf = nc.dram_tensor("slotbuf", [2, ntok], f32).ap()  # rank -> slots

    with tile.TileContext(nc) as tc, ExitStack() as ctx:
        cpool = ctx.enter_context(tc.tile_pool(name="const", bufs=1))
        xtp = ctx.enter_context(tc.tile_pool(name="xt", bufs=1))
        ctp = ctx.enter_context(tc.tile_pool(name="contrib", bufs=1))
        idxp = ctx.enter_context(tc.tile_pool(name="idx", bufs=1))

        # ---- constants ----
        gwsb = cpool.tile([P, CK * e], f32)
        nc.sync.dma_start(gwsb[:].rearrange("p (k e) -> p k e", e=e),
                          gw.rearrange("(k p) e -> p k e", p=P))
        trisb = cpool.tile([P, P], f32)
        nc.sync.dma_start(trisb[:], tri)
        id8sb = cpool.tile([e, e], f32)
        nc.sync.dma_start(id8sb[:], id8)
        gbsb = cpool.tile([e, 1], f32)
        nc.sync.dma_start(gbsb[:], gb)
        ebsb = cpool.tile([1, e], f32)
        nc.sync.dma_start(ebsb[:], ebase)
        ones1 = cpool.tile([1, P], f32)
        nc.vector.memset(ones1[:], 1.0)
        onescol = cpool.tile([P, 1], f32)
        nc.vector.memset(onescol[:], 1.0)
        iotaE_i = cpool.tile([P, TCH * e], dt.int32)
        nc.gpsimd.iota(iotaE_i[:], pattern=[[0, TCH], [1, e]], base=0,
                       channel_multiplier=0)
        iotaE = cpool.tile([P, TCH * e], f32)
        nc.vector.tensor_copy(iotaE[:], iotaE_i[:])
        toks_i = cpool.tile([P, TCH], dt.int32)
        nc.gpsimd.iota(toks_i[:], pattern=[[P, TCH]], base=0,
                       channel_multiplier=1)
        toksf = cpool.tile([P, TCH], f32)
        nc.vector.tensor_copy(toksf[:], toks_i[:])

        # ---- resident x (C-major) ----
        xts = []
        for k in range(CK):
            t = xtp.tile([P, ntok], f32, tag=f"xt{k}")
            nc.sync.dma_start(t[:], xT[k * P:(k + 1) * P, :])
            xts.append(t)

        # contrib buffers (expert outputs, gate-scaled, slot-major, C-major)
        contrib = ([ctp.tile([P, NSLOT], f32, tag=f"cb{cc}", name=f"cb{cc}")
                    for cc in range(CK)]
                   if level not in (40, 41, 42) else None)

        # combine gather indices (built in stage A)
        idx_all = [idxp.tile([P, ntok // 16], dt.int16, tag=f"ix{r}",
                             name=f"ix{r}") for r in range(2)]

        # =============== Stage A: gate + routing ===============
        with tc.tile_pool(name="sa", bufs=2) as sa, \
             tc.tile_pool(name="sa1", bufs=1) as sa1, \
             tc.tile_pool(name="pa", bufs=1, space="PSUM") as pa, \
             tc.tile_pool(name="pb", bufs=1, space="PSUM") as pb:
            # gate logits, expert-major: lgT[e, tok].
            # True fp32 matmul: fp32r is reduced-precision on HW and would
            # flip top-2 picks vs the reference on near-ties.
            lgT = sa1.tile([e, ntok], f32)
            lgps = [pa.tile([e, 512], f32, space="PSUM", tag=f"lg{nh}",
                            name=f"lg{nh}") for nh in range(NH)]
            for k in range(CK):
                for nh in range(NH):
                    nc.tensor.matmul(lgps[nh][:],
                                     lhsT=gwsb[:, k * e:(k + 1) * e],
                                     rhs=xts[k][:, nh * 512:(nh + 1) * 512],
                                     start=(k == 0), stop=(k == CK - 1))
            for nh in range(NH):
                nc.vector.tensor_scalar_add(lgT[:, nh * 512:(nh + 1) * 512],
                                            lgps[nh][:], gbsb[:, :1])
            # transpose to token-major [128, TCH, e]
            lg = sa1.tile([P, TCH, e], f32)
            for t in range(TCH):
                ps = pa.tile([P, e], f32, space="PSUM", tag="tp")
                nc.tensor.transpose(ps[:], lgT[:, t * P:(t + 1) * P], id8sb[:])
                nc.scalar.activation(lg[:, t, :], ps[:], AF.Copy)
            # softmax over experts
            mx = sa.tile([P, TCH], f32)
            nc.vector.tensor_reduce(mx[:], lg[:], axis=AX.X, op=ALU.max)
            xm = sa.tile([P, TCH, e], f32)
            nc.vector.tensor_tensor(out=xm[:], in0=lg[:],
                                    in1=mx[:].to_broadcast([P, TCH, e]),
                                    op=ALU.subtract)
            ex = sa.tile([P, TCH, e], f32)
            nc.scalar.activation(ex[:], xm[:], AF.Exp)
            sm = sa.tile([P, TCH], f32)
            nc.vector.tensor_reduce(sm[:], ex[:], axis=AX.X, op=ALU.add)
            rs = sa.tile([P, TCH], f32)
            nc.vector.reciprocal(rs[:], sm[:])
            probs = sa.tile([P, TCH, e], f32)
            nc.vector.tensor_tensor(out=probs[:], in0=ex[:],
                                    in1=rs[:].to_broadcast([P, TCH, e]),
                                    op=ALU.mult)
            # top-2 by logits (same order as by probs)
            mig = sa1.tile([P, TCH, 8], dt.uint32)
            for t in range(TCH):
                mv = sa.tile([P, 8], f32, tag="mv")
                nc.vector.max(mv[:], lg[:, t, :])
                nc.vector.max_index(mig[:, t, :], mv[:], lg[:, t, :])
            migf = sa1.tile([P, TCH, 8], f32)
            nc.vector.tensor_copy(migf[:], mig[:])

            A = []  # one-hot masks per rank [P, TCH, e]
            g = []  # gate values per rank [P, TCH]
            for r in range(2):
                Ar = sa1.tile([P, TCH, e], f32, tag=f"A{r}")
                nc.vector.tensor_tensor(
                    out=Ar[:], in0=migf[:, :, r:r + 1].to_broadcast([P, TCH, e]),
                    in1=iotaE[:].rearrange("p (t e) -> p t e", e=e),
                    op=ALU.is_equal)
                gr = sa1.tile([P, TCH], f32, tag=f"g{r}")
                tmp = sa.tile([P, TCH, e], f32, tag="gt")
                nc.vector.tensor_tensor(out=tmp[:], in0=probs[:], in1=Ar[:],
                                        op=ALU.mult)
                nc.vector.tensor_reduce(gr[:], tmp[:], axis=AX.X, op=ALU.add)
                A.append(Ar)
                g.append(gr)
            M = sa1.tile([P, TCH, e], f32)
            nc.vector.tensor_tensor(out=M[:], in0=A[0][:], in1=A[1][:],
                                    op=ALU.add)

            if level < 1:
                return nc
            # per-chunk prefix sums along tokens + running carry
            carry = sa1.tile([1, e], f32)
            nc.vector.memset(carry[:], 0.0)
            pssb = sa1.tile([P, TCH, e], f32)  # global slot+1 per (tok, e)
            for t in range(TCH):
                pf = pb.tile([P, e], f32, space="PSUM", tag="pf")
                nc.tensor.matmul(pf[:], lhsT=trisb[:], rhs=M[:, t, :],
                                 start=True, stop=True)
                bv = sa.tile([1, e], f32, tag="bv")
                nc.vector.tensor_tensor(out=bv[:], in0=ebsb[:], in1=carry[:],
                                        op=ALU.add)
                bb = pb.tile([P, e], f32, space="PSUM", tag="bb")
                nc.tensor.matmul(bb[:], lhsT=ones1[:], rhs=bv[:],
                                 start=True, stop=True)
                bbs = sa.tile([P, e], f32, tag="bbs")
                nc.scalar.activation(bbs[:], bb[:], AF.Copy)
                nc.vector.tensor_tensor(out=pssb[:, t, :], in0=pf[:],
                                        in1=bbs[:], op=ALU.add)
                totps = pb.tile([1, e], f32, space="PSUM", tag="tt")
                nc.tensor.matmul(totps[:], lhsT=onescol[:], rhs=M[:, t, :],
                                 start=True, stop=True)
                nc.vector.tensor_tensor(out=carry[:], in0=carry[:],
                                        in1=totps[:], op=ALU.add)

            slots_f = []
            slots_i = []
            for r in range(2):
                sel = sa.tile([P, TCH, e], f32, tag="sel")
                nc.vector.scalar_tensor_tensor(out=sel[:], in0=pssb[:],
                                               scalar=-1.0, in1=A[r][:],
                                               op0=ALU.add, op1=ALU.mult)
                sf = sa1.tile([P, TCH], f32, tag=f"sf{r}")
                nc.vector.tensor_reduce(sf[:], sel[:], axis=AX.X, op=ALU.add)
                si = sa1.tile([P, TCH], dt.int32, tag=f"si{r}")
                nc.vector.tensor_copy(si[:], sf[:])
                slots_f.append(sf)
                slots_i.append(si)
                # store token-order slots for the combine gather
                nc.sync.dma_start(
                    slotbuf[r, :].rearrange("(t p) -> p t", p=P), sf[:])

            if level < 2:
                return nc
            # meta prefill with zeros
            zt = sa.tile([P, NSLOT * 2 // P], f32, tag="zt")
            nc.vector.memset(zt[:], 0.0)
            nc.sync.dma_start(
                meta.rearrange("(p r) c -> p r c", p=P),
                zt[:].rearrange("p (r c) -> p r c", c=2))

            # scatter (token, gate) into slot-major meta
            for r in range(2):
                m_all = sa1.tile([P, TCH, 2], f32, tag=f"m{r}")
                nc.vector.tensor_copy(m_all[:, :, 0:1],
                                      toksf[:].rearrange("p (t o) -> p t o", o=1))
                nc.vector.tensor_copy(m_all[:, :, 1:2],
                                      g[r][:].rearrange("p (t o) -> p t o", o=1))
                for t in range(TCH):
                    nc.gpsimd.indirect_dma_start(
                        out=meta[:, :],
                        out_offset=bass.IndirectOffsetOnAxis(
                            ap=slots_i[r][:, t:t + 1], axis=0),
                        in_=m_all[:, t, :],
                        in_offset=None,
                        bounds_check=NSLOT - 1,
                        oob_is_err=False)

            if level < 3:
                return nc
            # combine gather indices in ap_gather wrap layout
            for r in range(2):
                tmpw = sa.tile([16, ntok // 16], f32, tag="tw")
                nc.sync.dma_start(
                    tmpw[:], slotbuf[r, :].rearrange("(f p) -> p f", p=16))
                w16 = sa.tile([16, ntok // 16], dt.int16, tag="w16")
                nc.vector.tensor_copy(w16[:], tmpw[:])
                for gidx in range(8):
                    nc.sync.dma_start(idx_all[r][gidx * 16:(gidx + 1) * 16, :],
                                      w16[:])

        if level < 4:
            return nc
        # =============== Stage B: expert MLP ===============
        with tc.tile_pool(name="mb", bufs=2) as mb, \
             tc.tile_pool(name="w1p", bufs=3) as w1p, \
             tc.tile_pool(name="w2p", bufs=3) as w2p, \
             tc.tile_pool(name="xgp", bufs=2) as xgp, \
             tc.tile_pool(name="hp", bufs=1) as hp, \
             tc.tile_pool(name="p1", bufs=2, space="PSUM") as p1, \
             tc.tile_pool(name="p2", bufs=1, space="PSUM") as p2, \
             tc.tile_pool(name="pg", bufs=2, space="PSUM") as pg:
            for ei in range(e):
                se = int(S[ei])
                b0 = int(base[ei])
                b1e = mb.tile([P, HK], f32, tag="b1e")
                nc.sync.dma_start(
                    b1e[:].rearrange("p (k o) -> p k o", o=1),
                    b1[ei].rearrange("(k p) one -> p k one", p=P))
                b2e = mb.tile([P, CK], f32, tag="b2e")
                nc.sync.dma_start(
                    b2e[:].rearrange("p (k o) -> p k o", o=1),
                    b2[ei].rearrange("(k p) one -> p k one", p=P))
                gat = mb.tile([1, se], f32, tag="gat")
                nc.sync.dma_start(gat[:],
                                  meta[b0:b0 + se, 1:2].rearrange("s one -> one s"))
                tw = mb.tile([16, se // 16], f32, tag="tw2")
                nc.sync.dma_start(
                    tw[:].rearrange("p (f o) -> p f o", o=1),
                    meta[b0:b0 + se, 0:1].rearrange("(f p) one -> p f one", p=16))
                t16s = mb.tile([16, se // 16], dt.int16, tag="t16s")
                nc.vector.tensor_copy(t16s[:], tw[:])
                t16 = mb.tile([P, se // 16], dt.int16, tag="t16")
                for gidx in range(8):
                    nc.sync.dma_start(t16[gidx * 16:(gidx + 1) * 16, :], t16s[:])

                for (woff, W) in _mlp_tiles(se):
                    iw = woff // 16
                    # gather x columns for this slot tile
                    xg = []
                    for k in range(CK):
                        xgf = xgp.tile([P, W], f32, tag=f"xgf{k}")
                        nc.gpsimd.ap_gather(
                            xgf[:], xts[k][:], t16[:, iw:iw + W // 16],
                            channels=P, num_elems=ntok, d=1, num_idxs=W)
                        xgt = xgp.tile([P, W], bf16, tag=f"xg{k}")
                        nc.vector.tensor_copy(xgt[:], xgf[:])
                        xg.append(xgt)
                    if level == 40:
                        for k in range(CK):
                            nc.gpsimd.dma_start(yT[k * P:(k + 1) * P, 0:W],
                                                xg[k][:])
                        continue
                    # layer 1
                    hs = []
                    for hk in range(HK):
                        wrow = w1p.tile([P, CK * P], bf16, tag="w1r")
                        nc.sync.dma_start(
                            wrow[:].rearrange("p (k h) -> p k h", k=CK),
                            w1[ei, :, hk * P:(hk + 1) * P].rearrange(
                                "(k p) h -> p k h", p=P))
                        ps = p1.tile([P, W], f32, space="PSUM", tag="ps1")
                        for k in range(CK):
                            nc.tensor.matmul(ps[:], lhsT=wrow[:, k * P:(k + 1) * P],
                                             rhs=xg[k][:],
                                             start=(k == 0), stop=(k == CK - 1))
                        ht = hp.tile([P, W], bf16, tag=f"h{hk}")
                        nc.scalar.activation(ht[:], ps[:], AF.Relu,
                                             bias=b1e[:, hk:hk + 1])
                        hs.append(ht)
                    if level == 41:
                        for hk in range(HK):
                            nc.gpsimd.dma_start(
                                yT[(hk % CK) * P:(hk % CK + 1) * P,
                                   (hk // CK) * W:(hk // CK) * W + W],
                                hs[hk][:])
                        continue
                    # gate broadcast for these slots
                    gps = pg.tile([P, W], f32, space="PSUM", tag="gps")
                    nc.tensor.matmul(gps[:], lhsT=ones1[:],
                                     rhs=gat[:, woff:woff + W],
                                     start=True, stop=True)
                    gbc = mb.tile([P, W], f32, tag="gbc")
                    nc.scalar.activation(gbc[:], gps[:], AF.Copy)
                    if level == 42:
                        nc.sync.dma_start(yT[0:P, 0:W], gbc[:])
                        for hk in range(HK):
                            nc.gpsimd.dma_start(
                                yT[(hk % CK) * P:(hk % CK + 1) * P,
                                   (hk // CK) * W:(hk // CK) * W + W],
                                hs[hk][:])
                        continue
                    # layer 2 in phases of up to 4 output chunks
                    for ch in range((CK + 3) // 4):
                        ncc = min(4, CK - ch * 4)
                        pss = [p2.tile([P, W], f32, space="PSUM",
                                       tag=f"ps2_{j}", name=f"ps2_{j}")
                               for j in range(ncc)]
                        for hk in range(HK):
                            w2t = w2p.tile([P, ncc * P], bf16, tag="w2t")
                            nc.sync.dma_start(
                                w2t[:, :ncc * P],
                                w2[ei, hk * P:(hk + 1) * P,
                                   ch * 4 * P:(ch * 4 + ncc) * P])
                            for j in range(ncc):
                                nc.tensor.matmul(
                                    pss[j][:], lhsT=w2t[:, j * P:(j + 1) * P],
                                    rhs=hs[hk][:],
                                    start=(hk == 0), stop=(hk == HK - 1))
                        for j in range(ncc):
                            cc = ch * 4 + j
                            nc.vector.scalar_tensor_tensor(
                                out=contrib[cc][:, b0 + woff:b0 + woff + W],
                                in0=pss[j][:], scalar=b2e[:, cc:cc + 1],
                                in1=gbc[:], op0=ALU.add, op1=ALU.mult)

        if level < 5 or level in (40, 41, 42):
            return nc
        # =============== Stage C: combine ===============
        with tc.tile_pool(name="cb", bufs=4) as cbp:
            for cc in range(CK):
                for nh in range(NH):
                    c1 = cbp.tile([P, 512], f32, tag="c1")
                    c2 = cbp.tile([P, 512], f32, tag="c2")
                    for r, ct in ((0, c1), (1, c2)):
                        nc.gpsimd.ap_gather(
                            ct[:], contrib[cc][:],
                            idx_all[r][:, nh * 32:(nh + 1) * 32],
                            channels=P, num_elems=NSLOT, d=1, num_idxs=512)
                    ys = cbp.tile([P, 512], f32, tag="ys")
                    nc.vector.tensor_tensor(out=ys[:], in0=c1[:], in1=c2[:],
                                            op=ALU.add)
                    nc.sync.dma_start(
                        yT[cc * P:(cc + 1) * P, nh * 512:(nh + 1) * 512], ys[:])

    return nc


# ---------------- host side ----------------

def _host_caps(xf, gate_w, gate_b, ntok=NTOK, margin=16):
    """Slot capacities per expert from a host replica of the routing."""
    logits = xf.astype(np.float32) @ gate_w.astype(np.float32) + gate_b
    order = np.argpartition(-logits, TOPK - 1, axis=1)[:, :TOPK]
    ncore = xf.shape[0] // ntok
    counts = np.zeros((ncore, E), np.int64)
    for cc in range(ncore):
        sl = order[cc * ntok:(cc + 1) * ntok]
        counts[cc] = np.bincount(sl.ravel(), minlength=E)
    maxc = counts.max(axis=0)
    S = ((maxc + margin + 15) // 16) * 16
    pad = (-int(S.sum())) % P
    S[-1] += pad
    return S.astype(np.int64)


def kernel(x, gate_w, gate_b, w1, b1, w2, b2):
    from concourse.bass_utils import run_bass_kernel_spmd

    x = np.asarray(x, np.float32)
    gate_w = np.asarray(gate_w, np.float32)
    gate_b = np.asarray(gate_b, np.float32)
    import ml_dtypes
    w1 = np.ascontiguousarray(np.asarray(w1).astype(ml_dtypes.bfloat16))
    b1 = np.asarray(b1, np.float32)
    w2 = np.ascontiguousarray(np.asarray(w2).astype(ml_dtypes.bfloat16))
    b2 = np.asarray(b2, np.float32)

    b, t, c = x.shape
    xf = x.reshape(b * t, c)
    S = _host_caps(xf, gate_w, gate_b)
    nc = build_program(S)

    ebase = np.concatenate([[0], np.cumsum(S)[:-1]]).astype(np.float32)
    shared = {
        "gw": gate_w,
        "gb": gate_b.reshape(E, 1).copy(),
        "w1": w1,
        "b1": b1.reshape(E, H, 1).copy(),
        "w2": w2,
        "b2": b2.reshape(E, C, 1).copy(),
        "tri": np.triu(np.ones((P, P), np.float32)),
        "id8": np.eye(E, dtype=np.float32),
        "ebase": ebase.reshape(1, E),
    }
    in_maps = []
    for cc in range(NCORE):
        sl = xf[cc * NTOK:(cc + 1) * NTOK]
        m = dict(shared)
        m["xT"] = np.ascontiguousarray(sl.T)
        in_maps.append(m)

    global LAST_BUILD
    LAST_BUILD = (nc, in_maps)
    res = run_bass_kernel_spmd(nc, in_maps, core_ids=list(range(NCORE)))
    outs = [np.ascontiguousarray(r["yT"].T) for r in res.results]
    y = np.concatenate(outs, axis=0).reshape(b, t, c)
    return y.astype(np.float32)
